# revision 1
# baseline (speedup 1.0000x reference)
"""Trainium2 Bass kernel for nn_CubeSimulator.

Reference computation: a 128^3 spatial grid is rotated (Rz(sky_rot) then
Rx(inclination)), a rotation-curve velocity field and an exponential-disk
intensity field are evaluated, an 80-channel Gaussian KDE over the
line-of-sight velocity reduces the third grid axis, and the [80,128,128]
cube is avg-pooled (5,4,4) to [16,32,32].

Kernel strategy
---------------
* Exact point-reflection symmetry: (i,j,k) -> (127-i,127-j,127-k) negates the
  rotated coordinates, so vz -> -vz and src is unchanged, giving
  cube[v, 127-i, 127-j] == cube[79-v, i, j] (the grid linspace is exactly
  antisymmetric in fp32).  Only the i < 64 half is computed on-device; the
  host mirrors the pooled output.  2x savings on everything.
* Sharding: the 64 computed sky-plane rows are split 8 rows/core over the 8
  NeuronCores (data-parallel over image rows, per the sharding hint).
* On-device layout: partitions = k (the reduced grid axis), free dims =
  (i_local=8) x (j=128) = 1024.  Per velocity channel the KDE summand is
  exp(L - (z_v - vz)^2/sig^2)  (intensity folded into the exponent), expanded
  as exp(a*z_v + b + c_v) with per-point a = 2 vz/sig^2,
  b = L - vz^2/sig^2 and per-channel c_v = -z_v^2/sig^2.  Inner loop:
    - one DVE scalar_tensor_tensor:  ARG = a*z_v + b
    - one ACT Exp (bias=c_v), emitting bf16 weights
    - PE matmuls against a ones-vector: sum over k (partitions) with
      channel-group accumulation in PSUM (the velocity avg-pool for free)
  Channels are processed in symmetric pairs (v, 79-v) which share c_v, so
  one ACT instruction covers both (large-N instructions amortize the ACT
  fixed overhead).
* All input-dependent scalars enter as DRAM tensors (per-partition operand
  columns), so the compiled program is input-independent and cached.
"""

import sys

for _p in ("/opt/trn_rl_repo",):
    if _p not in sys.path:
        sys.path.insert(0, _p)

import numpy as np
import ml_dtypes

# ---------------- problem constants (compile-time, model-intrinsic) --------
IMAGE_RES = 128          # internal spatial resolution
VEL_RES = 80             # internal velocity channels
VEL_UP = 5
IMG_UP = 4
N_CORES = 8
HALF_I = IMAGE_RES // 2          # 64 computed rows
ROWS_PER_CORE = HALF_I // N_CORES  # 8
FREE = ROWS_PER_CORE * IMAGE_RES   # 1024 free elements per partition
CUBE_FOV = 1000.0
M_TO_PC = 1.0 / 3.086e16
V_MAX_PC = np.float32(200000.0 * M_TO_PC)
R_C = np.float32(0.1 * CUBE_FOV)
R_D = np.float32(0.3 * CUBE_FOV)
H_Z = np.float32(0.05 * CUBE_FOV)
VEL_MIN = -300000.0
VEL_MAX = 300000.0

_INV_RD2 = 1.0 / (float(R_D) * float(R_D))  # Sqrt scale -> r2d/R_D
_EPS_R2D2 = np.float32(1e-25)  # host-folded guard for the reciprocal

# scalar-column layout inside the packed small input `sm`
# sm = [ nsz(1) | ciz(1) | zv2(80) | cv(40) | scal(8) ]
_C_NSZ = 0
_C_CIZ = 1
_C_ZV2 = 2                        # per-channel 2*z_v/sig^2
_C_CV = _C_ZV2 + VEL_RES          # 82
_C_SCAL = _C_CV + VEL_RES // 2    # 122
SM_COLS = _C_SCAL + 8             # 130
# scal sub-columns
_S_NSIG = _C_SCAL + 1    # -1/sig^2
_S_RC2 = _C_SCAL + 3     # R_C^2
_S_NEGH = _C_SCAL + 4    # -1/(2 H_Z^2)

_EARLY_SPLIT_PAIRS = 4   # pairs whose ACT op is halved to bridge startup

_CACHE = {}


def _build_program():
    from concourse import bacc, mybir, tile

    f32 = mybir.dt.float32
    bf16 = mybir.dt.bfloat16
    AF = mybir.ActivationFunctionType
    OP = mybir.AluOpType

    nc = bacc.Bacc(None)

    pk_d = nc.dram_tensor("pk", [128, 4 * FREE], f32, kind="ExternalInput")
    sm_d = nc.dram_tensor("sm", [128, SM_COLS], f32, kind="ExternalInput")
    ones_d = nc.dram_tensor("ones", [128, 64], bf16, kind="ExternalInput")
    out_d = nc.dram_tensor("out", [16, 1024], f32, kind="ExternalOutput")

    with tile.TileContext(nc) as tc:
        with (
            tc.tile_pool(name="inp", bufs=1) as inp,
            tc.tile_pool(name="fld", bufs=1) as fld,
            tc.tile_pool(name="arg", bufs=6) as argp,
            tc.tile_pool(name="wp", bufs=6) as wp,
            tc.tile_pool(name="psum", bufs=6, space="PSUM") as psum,
            tc.tile_pool(name="ob", bufs=4) as obp,
        ):
            pk = inp.tile([128, 4 * FREE], f32)
            sm = inp.tile([128, SM_COLS], f32)
            ones = inp.tile([128, 64], bf16)
            # small inputs ride the gpsimd SWDGE queue so the big pk
            # chunks start immediately on the sync queue
            nc.gpsimd.dma_start(sm[:], sm_d[:])
            nc.gpsimd.dma_start(ones[:], ones_d[:])
            # 256KB chunks ordered by when the field chains need them:
            # pa/pb/prx2 halves feed the chain heads, pc only at vzt
            H2 = FREE // 2
            for _c in (0, 2, 4, 1, 3, 5, 6, 7):
                nc.sync.dma_start(pk[:, _c * H2:(_c + 1) * H2],
                                  pk_d[:, _c * H2:(_c + 1) * H2])

            pa = pk[:, 0 * FREE:1 * FREE]
            pb = pk[:, 1 * FREE:2 * FREE]
            prx2 = pk[:, 2 * FREE:3 * FREE]
            pc = pk[:, 3 * FREE:4 * FREE]

            def col(i):
                return sm[:, i:i + 1]

            # ---- field: vz and b = L - vz^2/sig^2, in two 512 halves so the
            # KDE pipeline can start on half 0 while half 1 is in flight ----
            y2 = fld.tile([128, FREE], f32)
            r2d2 = fld.tile([128, FREE], f32)
            z2 = fld.tile([128, FREE], f32)
            q = fld.tile([128, FREE], f32)
            den = fld.tile([128, FREE], f32)
            rec = fld.tile([128, FREE], f32)
            u = fld.tile([128, FREE], f32)
            su = fld.tile([128, FREE], f32)
            vzt = fld.tile([128, FREE], f32)
            nvs = fld.tile([128, FREE], f32)
            slq = fld.tile([128, FREE], f32)
            t2 = fld.tile([128, FREE], f32)
            bb = fld.tile([128, FREE], f32)
            scratch = fld.tile([128, FREE], f32)
            qc = fld.tile([128, FREE], f32)

            V = nc.vector
            G = nc.gpsimd
            HALF = FREE // 2
            # half 0 on DVE, half 1 on gpsimd: the two chains run in
            # parallel, halving time-to-first-Exp (reciprocal is DVE-only).
            # Emission staged so both halves' reciprocals get early DVE
            # priority.
            CHUNKS = ((0, V), (1, G))

            def sl_of(h):
                return slice(h * HALF, (h + 1) * HALF)

            roty = fld.tile([128, FREE], f32)
            rotz = fld.tile([128, FREE], f32)
            for c, E in CHUNKS:
                s = sl_of(c)
                # rot_y = A + (-si*z_k); rot_z = B + (ci*z_k)
                E.tensor_scalar_add(roty[:, s], pa[:, s], col(_C_NSZ))
                E.tensor_scalar_add(rotz[:, s], pb[:, s], col(_C_CIZ))
                E.tensor_mul(y2[:, s], roty[:, s], roty[:, s])
                E.tensor_add(r2d2[:, s], y2[:, s], prx2[:, s])  # rx2 has +eps
                E.tensor_mul(z2[:, s], rotz[:, s], rotz[:, s])
                E.tensor_add(q[:, s], r2d2[:, s], z2[:, s])
                # den = (q + R_C^2) * r2d2  (Pool stt only supports
                # (mult, add); decompose on the gpsimd chunks)
                if E is V:
                    E.scalar_tensor_tensor(den[:, s], q[:, s], col(_S_RC2),
                                           r2d2[:, s], op0=OP.add, op1=OP.mult)
                else:
                    E.tensor_scalar_add(qc[:, s], q[:, s], col(_S_RC2))
                    E.tensor_mul(den[:, s], qc[:, s], r2d2[:, s])
            for c, _E in CHUNKS:
                s = sl_of(c)
                V.reciprocal_approx_accurate(rec[:, s], den[:, s],
                                             scratch[:, s])
            for c, E in CHUNKS:
                s = sl_of(c)
                E.tensor_mul(u[:, s], q[:, s], rec[:, s])
                nc.scalar.activation(su[:, s], u[:, s], AF.Sqrt)
                E.tensor_mul(vzt[:, s], su[:, s], pc[:, s])
                # nvs = (vz * -1/sig^2) * vz
                if E is V:
                    E.scalar_tensor_tensor(nvs[:, s], vzt[:, s], col(_S_NSIG),
                                           vzt[:, s], op0=OP.mult, op1=OP.mult)
                else:
                    E.tensor_scalar_mul(qc[:, s], vzt[:, s], col(_S_NSIG))
                    E.tensor_mul(nvs[:, s], qc[:, s], vzt[:, s])
                # slq = sqrt(r2d2 / R_D^2)
                nc.scalar.activation(slq[:, s], r2d2[:, s], AF.Sqrt,
                                     scale=_INV_RD2)
                # t2 = z2 * (-1/(2 H_Z^2)) + nvs ;  b = t2 - slq
                if E is V:
                    E.scalar_tensor_tensor(t2[:, s], z2[:, s], col(_S_NEGH),
                                           nvs[:, s], op0=OP.mult, op1=OP.add)
                else:
                    E.tensor_scalar_mul(qc[:, s], z2[:, s], col(_S_NEGH))
                    E.tensor_add(t2[:, s], qc[:, s], nvs[:, s])
                E.tensor_sub(bb[:, s], t2[:, s], slq[:, s])

            # ---- KDE: symmetric channel pairs (v, 79-v) ----
            psum_tiles = {}
            grp_count = {}
            # Channel pairs (v, 79-v) have exactly opposite z_v (the host
            # forces zv2 antisymmetric), so arg_{79-v} = 2*b - arg_v
            # (2*b is exact in fp32).  Per-pair engine configs, greedily
            # balanced (cost-model ns):
            #   stt: both channels via stt on DVE
            #   mix: arg_v stt on DVE, arg_{79-v} = bb2 - arg_v on gpsimd
            #   gp:  m = vzt*zv2 ; arg_v = m + b ; arg_{79-v} = bb2 - arg_v
            bb2 = fld.tile([128, FREE], f32)
            for c, E in CHUNKS:
                s = sl_of(c)
                E.tensor_add(bb2[:, s], bb[:, s], bb[:, s])
            eng_t = {"dve": 20_400.0, "gp": 7_900.0}
            CFG = [(2224, 0, "stt"), (1112, 853, "mix"), (0, 2559, "gp")]
            for v in range(VEL_RES // 2):
                vm = VEL_RES - 1 - v
                split = v < _EARLY_SPLIT_PAIRS or v == VEL_RES // 2 - 1
                arg = argp.tile([128, 2 * FREE], f32, tag="arg")
                if split:
                    # gpsimd is still busy with the half-1 field chain at
                    # startup; keep the early pairs entirely on DVE
                    best = CFG[0]
                else:
                    best = min(CFG, key=lambda c: max(eng_t["dve"] + c[0],
                                                      eng_t["gp"] + c[1]))
                eng_t["dve"] += best[0]
                eng_t["gp"] += best[1]
                mode = best[2]
                mt = None
                if mode != "stt":
                    mt = argp.tile([128, FREE], f32, tag="mt", bufs=2)
                w = wp.tile([128, 2 * FREE], bf16, tag="w")

                def emit_args(fs, asl_v, asl_m):
                    """fs: field slice; asl_v/asl_m: arg slices for v, 79-v"""
                    if mode == "stt":
                        V.scalar_tensor_tensor(
                            arg[:, asl_v], vzt[:, fs], col(_C_ZV2 + v),
                            bb[:, fs], op0=OP.mult, op1=OP.add)
                        V.scalar_tensor_tensor(
                            arg[:, asl_m], vzt[:, fs], col(_C_ZV2 + vm),
                            bb[:, fs], op0=OP.mult, op1=OP.add)
                    elif mode == "mix":
                        V.scalar_tensor_tensor(
                            arg[:, asl_v], vzt[:, fs], col(_C_ZV2 + v),
                            bb[:, fs], op0=OP.mult, op1=OP.add)
                        G.tensor_sub(arg[:, asl_m], bb2[:, fs], arg[:, asl_v])
                    else:
                        G.tensor_scalar_mul(mt[:, fs], vzt[:, fs],
                                            col(_C_ZV2 + v))
                        G.tensor_add(arg[:, asl_v], mt[:, fs], bb[:, fs])
                        G.tensor_sub(arg[:, asl_m], bb2[:, fs], arg[:, asl_v])

                if split:
                    # layout [ch0h0|ch1h0|ch0h1|ch1h1]: Exp on half 0 can run
                    # before the field finishes half 1
                    for hq in range(2):
                        fs = sl_of(hq)
                        emit_args(fs,
                                  slice(2 * hq * HALF, (2 * hq + 1) * HALF),
                                  slice((2 * hq + 1) * HALF,
                                        (2 * hq + 2) * HALF))
                        nc.scalar.activation(
                            w[:, 2 * hq * HALF:2 * (hq + 1) * HALF],
                            arg[:, 2 * hq * HALF:2 * (hq + 1) * HALF],
                            AF.Exp, bias=col(_C_CV + v))
                else:
                    emit_args(slice(0, FREE), slice(0, FREE),
                              slice(FREE, 2 * FREE))
                    nc.scalar.activation(w[:], arg[:], AF.Exp,
                                         bias=col(_C_CV + v))

                for hh, ch in enumerate((v, vm)):
                    vo = ch // VEL_UP
                    if vo not in psum_tiles:
                        # one bank; halves land on partition rows 0 and 64
                        # so the PSUM->SBUF copy reads 512/partition, not
                        # 1024 (matmul out base must be 0/32/64)
                        psum_tiles[vo] = psum.tile([128, HALF], f32,
                                                   tag="acc", name=f"acc{vo}")
                        grp_count[vo] = 0
                    pt = psum_tiles[vo]
                    cnt = grp_count[vo]
                    if split:
                        mm = [((2 * ck + hh) * HALF, (2 * ck + hh + 1) * HALF,
                               ck, 0, HALF) for ck in range(2)]
                    else:
                        mm = [(hh * FREE + ck * HALF,
                               hh * FREE + (ck + 1) * HALF, ck, 0, HALF)
                              for ck in range(2)]
                    for w0, w1, rb, o0, o1 in mm:
                        nc.tensor.matmul(
                            pt[64 * rb:64 * rb + 64, o0:o1], ones[:, :],
                            w[:, w0:w1],
                            start=(cnt == 0), stop=(cnt == VEL_UP - 1),
                            # rows 0-63 and 64-127 are separate groups on HW;
                            # CoreSim's zero-region check ignores the
                            # partition base and false-positives
                            skip_group_check=True,
                        )
                    grp_count[vo] = cnt + 1
                    if grp_count[vo] == VEL_UP:
                        # v-pooled cube rows; (i,j) spatial pooling + scaling
                        # happens on the host.  DMA cannot read PSUM and
                        # compute APs need partition step 1, so copy the
                        # contiguous [65, 512] block (cost ~ free size) and
                        # let the DMA pick rows 0 and 64.  The very last
                        # completion copies via the then-idle ACT so the two
                        # final copies run in parallel.
                        ot = obp.tile([65, HALF], f32, tag="ob",
                                      name=f"ot{vo}")
                        if v == VEL_RES // 2 - 1 and hh == 1:
                            nc.scalar.activation(ot[:, :], pt[0:65, :],
                                                 AF.Copy)
                        else:
                            V.tensor_copy(ot[:, :], pt[0:65, :])
                        nc.sync.dma_start(
                            out_d[vo, :].rearrange("(q n) -> q n", q=2),
                            ot[0:65:64, :])
                        del psum_tiles[vo]

    nc.finalize()  # Bacc: runs compile() passes (wait splitting, reg alloc)
    return nc


def _host_inputs(inclination, sky_rot, line_broadening):
    f32 = np.float32
    inc = f32(inclination)
    rot = f32(sky_rot)
    lb = f32(line_broadening)
    ci, si = f32(np.cos(inc)), f32(np.sin(inc))
    cr, sr = f32(np.cos(rot)), f32(np.sin(rot))
    sig_sq = f32(lb * lb)

    lin = np.linspace(-CUBE_FOV, CUBE_FOV, IMAGE_RES, dtype=f32)
    z_labels = np.linspace(f32(VEL_MIN * M_TO_PC), f32(VEL_MAX * M_TO_PC),
                           VEL_RES, dtype=f32)

    sm = np.zeros((128, SM_COLS), dtype=f32)
    sm[:, _C_NSZ] = (-si * lin).astype(f32)          # -si * z_k
    sm[:, _C_CIZ] = (ci * lin).astype(f32)           # ci * z_k
    # 2*z_v/sig^2, matching fp32 eval order z_v * (2/sig^2); forced exactly
    # antisymmetric (z_labels is antisymmetric to 1 ulp) so the device can
    # compute arg_{79-v} = b - m from m = vzt*zv2_v
    zv2 = (z_labels * f32(2.0 / sig_sq)).astype(f32)
    zv2[VEL_RES // 2:] = -zv2[:VEL_RES // 2][::-1]
    sm[:, _C_ZV2:_C_ZV2 + VEL_RES] = zv2
    cvv = (-(z_labels[:40] * z_labels[:40]) / sig_sq).astype(f32)
    sm[:, _C_CV:_C_CV + 40] = cvv
    sm[:, _S_NSIG] = f32(-1.0 / sig_sq)
    sm[:, _S_RC2] = f32(float(R_C) * float(R_C))
    sm[:, _S_NEGH] = f32(-1.0 / (2.0 * float(H_Z) * float(H_Z)))
    ones = np.ones((128, 64), dtype=ml_dtypes.bfloat16)

    in_maps = []
    for c in range(N_CORES):
        x = lin[8 * c: 8 * c + 8][:, None]                 # [8,1]
        y = lin[None, :]                                   # [1,128]
        y1 = (sr * x + cr * y).astype(f32)
        A = (ci * y1).astype(f32).reshape(-1)
        B = (si * y1).astype(f32).reshape(-1)
        rot_x = (cr * x - sr * y).astype(f32)
        rx2 = (rot_x * rot_x + _EPS_R2D2).astype(f32).reshape(-1)
        C = (-si * V_MAX_PC * rot_x).astype(f32).reshape(-1)
        pkrow = np.concatenate([A, B, rx2, C]).astype(f32)  # [4*FREE]
        pk = np.ascontiguousarray(np.broadcast_to(pkrow, (128, 4 * FREE)))
        in_maps.append({"pk": pk, "sm": sm, "ones": ones})
    return in_maps


def _run(in_maps, trace=False, **kwargs):
    from concourse.bass_utils import run_bass_kernel_spmd
    if "nc" not in _CACHE:
        _CACHE["nc"] = _build_program()
    return run_bass_kernel_spmd(_CACHE["nc"], in_maps,
                                list(range(N_CORES)), trace=trace, **kwargs)


def _assemble(results, line_broadening):
    f32 = np.float32
    lb = f32(line_broadening)
    sig_sq = f32(lb * lb)
    pref = f32(1.0 / np.sqrt(2.0 * np.pi * sig_sq))
    scale = f32(pref / f32(VEL_UP * IMG_UP * IMG_UP))
    parts = []
    for r in results:
        cube = np.asarray(r["out"]).reshape(16, 2, 4, 32, 4)  # vo,io,di,jo,dj
        pooled = cube.sum(axis=(2, 4), dtype=np.float32) * scale  # [16,2,32]
        parts.append(pooled.astype(f32))
    half = np.concatenate(parts, axis=1)
    full = np.empty((16, 32, 32), dtype=np.float32)
    full[:, :16, :] = half
    full[:, 16:, :] = half[::-1, ::-1, ::-1]
    return full


def kernel(inclination, sky_rot, line_broadening):
    in_maps = _host_inputs(inclination, sky_rot, line_broadening)
    res = _run(in_maps)
    return _assemble(res.results, line_broadening)



# revision 12
# speedup vs baseline: 3.6005x; 3.6005x over previous
"""Trainium2 Bass kernel for nn_CubeSimulator.

Reference computation: a 128^3 spatial grid is rotated (Rz(sky_rot) then
Rx(inclination)), a rotation-curve velocity field and an exponential-disk
intensity field are evaluated, an 80-channel Gaussian KDE over the
line-of-sight velocity reduces the third grid axis, and the [80,128,128]
cube is avg-pooled (5,4,4) to [16,32,32].

Kernel strategy (v2)
--------------------
* erf-collapsed KDE: the output only needs 5-channel pooled sums.  A
  5-channel group sum of Gaussians at spacing dz << sigma is a midpoint-rule
  sum, equal to an erf difference with super-exponentially small aliasing
  error once the erf width is corrected to sig_e = sqrt(sig^2 - dz^2/6):
      sum_{r=0..4} exp(-(z_{5m+r}-vz)^2/sig^2)
        ~= C * [erf((e_{m+1}-vz)/sig_e) - erf((e_m-vz)/sig_e)],
  where e_m are the 17 group edges and C = sqrt(pi)*sig/(2 dz).  Max error
  ~3e-5 per group (values up to ~3.9).  80 exp-channels collapse to 15
  erf evaluations (the outermost edges saturate to +-1 for physical |vz|).
* +- PE accumulation: per edge only P_m = erf_m * src is formed; PSUM group
  m accumulates (+P_{m+1}) and (-P_m) via +1/-1 stationary vectors, so no
  elementwise differences are needed.
* k-window packing: src has a Gaussian vertical profile; per sky column the
  |rot_z| <= 32*ci*dz_grid window (64 k-steps) holds all non-negligible
  intensity.  Two sky points (same output pool cell) pack into one
  128-partition column -> half the free size everywhere.
* Point symmetry: (i,j,k) -> (-i,-j,-k) negates vz and preserves src, so
  only rows i<64 are computed; the host mirrors the pooled output.
* All input-dependent scalars enter via DRAM tensors (per-partition operand
  columns), so the compiled program is input-independent and cached.
"""

import sys

for _p in ("/opt/trn_rl_repo",):
    if _p not in sys.path:
        sys.path.insert(0, _p)

import numpy as np
import ml_dtypes

# ---------------- problem constants (compile-time, model-intrinsic) --------
IMAGE_RES = 128
VEL_RES = 80
VEL_UP = 5
IMG_UP = 4
N_CORES = 8
HALF_I = IMAGE_RES // 2            # 64 computed rows
ROWS_PER_CORE = HALF_I // N_CORES  # 8
KWIN = 64                          # k-window length (2 points/column)
COLS = ROWS_PER_CORE * IMAGE_RES // 2   # 512 packed columns per core
CUBE_FOV = 1000.0
M_TO_PC = 1.0 / 3.086e16
V_MAX_PC = np.float32(200000.0 * M_TO_PC)
R_C = np.float32(0.1 * CUBE_FOV)
R_D = np.float32(0.3 * CUBE_FOV)
H_Z = np.float32(0.05 * CUBE_FOV)
VEL_MIN = -300000.0
VEL_MAX = 300000.0
N_GROUPS = VEL_RES // VEL_UP       # 16
N_EDGES = N_GROUPS + 1             # 17

_INV_RD2 = 1.0 / (float(R_D) * float(R_D))
_NEG_H = float(-1.0 / (2.0 * float(H_Z) * float(H_Z)))
_RC2 = float(R_C) * float(R_C)
_EPS_R2D2 = np.float32(1e-25)

# sm scalar-column layout: [ nsz | ciz | e'_1..e'_15 | rc2 | negh ]
_C_NSZ = 0
_C_CIZ = 1
_C_EDG = 2                 # e'_m for m=1..15 at columns 2..16
_C_RC2 = 17
_C_NEGH = 18
SM_COLS = 19

_CACHE = {}


def _build_program():
    from concourse import bacc, mybir, tile

    f32 = mybir.dt.float32
    bf16 = mybir.dt.bfloat16
    AF = mybir.ActivationFunctionType
    OP = mybir.AluOpType

    nc = bacc.Bacc(None)

    pk_d = nc.dram_tensor("pk", [128, 4 * COLS], f32, kind="ExternalInput")
    sm_d = nc.dram_tensor("sm", [128, SM_COLS], f32, kind="ExternalInput")
    ones_d = nc.dram_tensor("ones", [128, 64], bf16, kind="ExternalInput")
    out_d = nc.dram_tensor("out", [16, COLS], f32, kind="ExternalOutput")

    with tile.TileContext(nc) as tc:
        with (
            tc.tile_pool(name="inp", bufs=1) as inp,
            tc.tile_pool(name="fld", bufs=1) as fld,
            tc.tile_pool(name="ep", bufs=3) as ep,
            tc.tile_pool(name="pp", bufs=4) as pp,
            tc.tile_pool(name="psum", bufs=1, space="PSUM") as psum,
            tc.tile_pool(name="ob", bufs=4) as obp,
        ):
            pk = inp.tile([128, 4 * COLS], f32)
            sm = inp.tile([128, SM_COLS], f32)
            ones = inp.tile([128, 64], bf16)
            nc.gpsimd.dma_start(sm[:], sm_d[:])
            nc.gpsimd.dma_start(ones[:], ones_d[:])
            # order: pa, pb (field chain heads), rx2, pc
            for _c in (0, 1, 2, 3):
                nc.sync.dma_start(pk[:, _c * COLS:(_c + 1) * COLS],
                                  pk_d[:, _c * COLS:(_c + 1) * COLS])

            pa = pk[:, 0 * COLS:1 * COLS]
            pb = pk[:, 1 * COLS:2 * COLS]
            prx2 = pk[:, 2 * COLS:3 * COLS]
            pc = pk[:, 3 * COLS:4 * COLS]

            def col(i):
                return sm[:, i:i + 1]

            V = nc.vector
            G = nc.gpsimd
            S = nc.scalar

            roty = fld.tile([128, COLS], f32)
            rotz = fld.tile([128, COLS], f32)
            y2 = fld.tile([128, COLS], f32)
            r2d2 = fld.tile([128, COLS], f32)
            z2 = fld.tile([128, COLS], f32)
            q = fld.tile([128, COLS], f32)
            qc = fld.tile([128, COLS], f32)
            den = fld.tile([128, COLS], f32)
            rec = fld.tile([128, COLS], f32)
            scratch = fld.tile([128, COLS], f32)
            u = fld.tile([128, COLS], f32)
            su = fld.tile([128, COLS], f32)
            slq = fld.tile([128, COLS], f32)
            sarg = fld.tile([128, COLS], f32)
            vzt = fld.tile([128, COLS], f32)
            src = fld.tile([128, COLS], bf16)
            dummy = fld.tile([128, COLS], bf16)

            # PE warmup: keep the tensor engine streaming during the field
            # phase so the p-state ramp reaches full clock by the KDE.
            G.memset(dummy[:], 0.0)

            HALF = COLS // 2
            # chunk 0 on gpsimd(Pool), chunk 1 on DVE
            CHUNKS = ((0, G), (1, V))

            def sl_of(c):
                return slice(c * HALF, (c + 1) * HALF)

            for c, E in CHUNKS:
                s = sl_of(c)
                E.tensor_scalar_add(roty[:, s], pa[:, s], col(_C_NSZ))
                E.tensor_scalar_add(rotz[:, s], pb[:, s], col(_C_CIZ))
                E.tensor_mul(y2[:, s], roty[:, s], roty[:, s])
                E.tensor_add(r2d2[:, s], y2[:, s], prx2[:, s])
                E.tensor_mul(z2[:, s], rotz[:, s], rotz[:, s])
                E.tensor_add(q[:, s], r2d2[:, s], z2[:, s])
                if E is V:
                    E.scalar_tensor_tensor(den[:, s], q[:, s], col(_C_RC2),
                                           r2d2[:, s], op0=OP.add, op1=OP.mult)
                else:
                    E.tensor_scalar_add(qc[:, s], q[:, s], col(_C_RC2))
                    E.tensor_mul(den[:, s], qc[:, s], r2d2[:, s])

            # slq = sqrt(r2d2)/R_D (full width; r2d2 ready before u)
            S.activation(slq[:], r2d2[:], AF.Sqrt, scale=_INV_RD2)

            # (warmup matmuls target the last psum bank, which is only
            # needed for real accumulation at the very end)

            for c, _E in CHUNKS:
                s = sl_of(c)
                V.reciprocal_approx_accurate(rec[:, s], den[:, s],
                                             scratch[:, s])
            for c, E in CHUNKS:
                s = sl_of(c)
                E.tensor_mul(u[:, s], q[:, s], rec[:, s])

            # su = sqrt(u) (full width)
            S.activation(su[:], u[:], AF.Sqrt)

            for c, E in CHUNKS:
                s = sl_of(c)
                E.tensor_mul(vzt[:, s], su[:, s], pc[:, s])
                # sarg = z2*(-1/(2 H_Z^2)) - slq
                if E is V:
                    E.scalar_tensor_tensor(sarg[:, s], z2[:, s], col(_C_NEGH),
                                           slq[:, s], op0=OP.mult,
                                           op1=OP.subtract)
                else:
                    E.tensor_scalar_mul(qc[:, s], z2[:, s], col(_C_NEGH))
                    E.tensor_sub(sarg[:, s], qc[:, s], slq[:, s])

            # src = exp(sarg) -> bf16 (exp table)
            S.activation(src[:], sarg[:], AF.Exp)

            # ---- KDE: 15 erf edges + (+-1)-stationary PSUM accumulation --
            # group m (psum bank m//4, partition row 32*(m%4)):
            #   S_m = P_{m+1} - P_m,  P_m = erf((e_m - vz)/sig_e) * src,
            #   P_0 := -src (E_0 = -1), P_16 := +src (E_16 = +1).
            onesp = ones[:, 0:32]
            onesn = ones[:, 32:64]
            # 3 groups per bank at partition bases 0/32/64 (matmul API
            # restriction); 6 banks for 16 groups.
            psb = [psum.tile([128, COLS], f32, name=f"acc{b}")
                   for b in range(6)]

            # warmup streams into the last bank (first really used at m=15)
            for _w in range(4):
                nc.tensor.matmul(psb[5][0:32, :], ones[:, 0:32],
                                 dummy[:, :], start=True, stop=True,
                                 skip_group_check=True)

            def mm(m, stat, mov, start, stop):
                b, g = m // 3, m % 3
                nc.tensor.matmul(psb[b][32 * g:32 * g + 32, :], stat, mov,
                                 start=start, stop=stop,
                                 skip_group_check=True)

            def bank_done(b):
                ngr = 3 if b < 5 else 1
                ot = obp.tile([128, COLS], f32, tag="ob", name=f"ot{b}")
                V.tensor_copy(ot[0:32 * ngr, :], psb[b][0:32 * ngr, :])
                nc.sync.dma_start(
                    out_d[3 * b:3 * b + ngr, :],
                    ot[0:32 * ngr:32, :])

            mm(0, onesp, src[:, :], True, False)       # +src into group 0
            for m in range(1, 16):
                E_t = ep.tile([128, COLS], bf16, tag="E")
                S.activation(E_t[:], vzt[:], AF.Erf, bias=col(_C_EDG + m - 1),
                             scale=-1.0)
                P_t = pp.tile([128, COLS], bf16, tag="P")
                # split the 15 products between DVE and Pool
                (V if m % 2 else G).tensor_mul(P_t[:], E_t[:], src[:])
                mm(m - 1, onesp, P_t[:, :], False, True)
                mm(m, onesn, P_t[:, :], True, False)
                if (m - 1) % 3 == 2:
                    bank_done((m - 1) // 3)
            mm(15, onesp, src[:, :], False, True)      # +src into group 15
            bank_done(5)

    nc.finalize()
    return nc


def _host_inputs(inclination, sky_rot, line_broadening):
    f32 = np.float32
    inc = f32(inclination)
    rot = f32(sky_rot)
    lb = f32(line_broadening)
    ci, si = f32(np.cos(inc)), f32(np.sin(inc))
    cr, sr = f32(np.cos(rot)), f32(np.sin(rot))

    lin = np.linspace(-CUBE_FOV, CUBE_FOV, IMAGE_RES, dtype=f32)
    dgrid = f32(lin[1] - lin[0])
    zl = np.linspace(f32(VEL_MIN * M_TO_PC), f32(VEL_MAX * M_TO_PC),
                     VEL_RES, dtype=f32)
    dz = float(zl[-1] - zl[0]) / (VEL_RES - 1)
    sig = float(lb)
    sig_e = f32(np.sqrt(sig * sig - dz * dz / 6.0))

    # packing validity: dropped vertical mass outside the 64-step window
    t_keep = (KWIN / 2) * abs(float(ci)) * float(dgrid)
    eps_drop = np.exp(-t_keep * t_keep / (2.0 * float(H_Z) ** 2))
    if eps_drop > 1e-4:
        raise RuntimeError(
            f"k-window packing invalid for inclination={inc} "
            f"(eps_drop={eps_drop:.2e}); phi=1 fallback not built")

    # group edges, antisymmetric
    edges = np.empty(N_EDGES, dtype=np.float64)
    edges[0] = zl[0] - dz / 2
    for m in range(1, N_GROUPS):
        edges[m] = (float(zl[5 * m - 1]) + float(zl[5 * m])) / 2
    edges[N_GROUPS] = zl[-1] + dz / 2
    edges = (edges - edges[::-1]) / 2

    # outer-edge saturation check (E_0 = -1, E_16 = +1)
    vmax_proj = abs(float(si)) * float(V_MAX_PC)
    margin = (float(edges[N_GROUPS]) - vmax_proj) / float(sig_e)
    if margin < 4.5:
        raise RuntimeError(
            f"outer erf edge not saturated (margin={margin:.2f} sigma); "
            "const-edge variant not built")

    sm = np.zeros((128, SM_COLS), dtype=f32)
    pmod = (np.arange(128) % KWIN).astype(f32)
    sm[:, _C_NSZ] = (-si * dgrid) * pmod
    sm[:, _C_CIZ] = (ci * dgrid) * pmod
    for m in range(1, N_GROUPS):
        sm[:, _C_EDG + m - 1] = f32(edges[m] / sig_e)
    sm[:, _C_RC2] = f32(_RC2)
    sm[:, _C_NEGH] = f32(_NEG_H)

    ones = np.empty((128, 64), dtype=ml_dtypes.bfloat16)
    ones[:, 0:32] = 1.0
    ones[:, 32:64] = -1.0

    in_maps = []
    for core in range(N_CORES):
        xs = lin[ROWS_PER_CORE * core: ROWS_PER_CORE * (core + 1)]
        # half-column order: for each cell (iob, jo): 16 points in
        # (di, dj) raster order; consecutive pairs share a packed column.
        il = np.empty(2 * COLS, dtype=np.int64)
        jj = np.empty(2 * COLS, dtype=np.int64)
        idx = 0
        for iob in range(2):
            for jo in range(32):
                for di in range(4):
                    for dj in range(4):
                        il[idx] = iob * 4 + di
                        jj[idx] = jo * 4 + dj
                        idx += 1
        x = xs[il].astype(f32)
        y = lin[jj].astype(f32)
        y1 = (sr * x + cr * y).astype(f32)
        rotx = (cr * x - sr * y).astype(f32)
        # k-window start per half-column
        kc = (-si * y1 / ci - lin[0]) / dgrid
        k0 = np.clip(np.round(kc - KWIN / 2), 0, IMAGE_RES - KWIN
                     ).astype(np.int64)
        zk0 = (lin[0] + k0.astype(f32) * dgrid).astype(f32)
        A = (ci * y1 - si * zk0).astype(f32)       # + nsz ramp -> rot_y
        B = (si * y1 + ci * zk0).astype(f32)       # + ciz ramp -> rot_z
        rx2 = (rotx * rotx + _EPS_R2D2).astype(f32)
        C = (-si * V_MAX_PC * rotx / sig_e).astype(f32)

        # pack: half-column 2c -> partitions 0..63, 2c+1 -> 64..127
        pk = np.empty((128, 4 * COLS), dtype=f32)
        for t, arr in enumerate((A, B, rx2, C)):
            pk[:64, t * COLS:(t + 1) * COLS] = arr[0::2][None, :]
            pk[64:, t * COLS:(t + 1) * COLS] = arr[1::2][None, :]
        in_maps.append({"pk": pk, "sm": sm, "ones": ones})
    return in_maps


def _run(in_maps, trace=False, **kwargs):
    from concourse.bass_utils import run_bass_kernel_spmd
    if "nc" not in _CACHE:
        _CACHE["nc"] = _build_program()
    return run_bass_kernel_spmd(_CACHE["nc"], in_maps,
                                list(range(N_CORES)), trace=trace, **kwargs)


def _assemble(results, line_broadening):
    f32 = np.float32
    lb = f32(line_broadening)
    sig = float(lb)
    zl = np.linspace(f32(VEL_MIN * M_TO_PC), f32(VEL_MAX * M_TO_PC),
                     VEL_RES, dtype=f32)
    dz = float(zl[-1] - zl[0]) / (VEL_RES - 1)
    cmag = np.sqrt(np.pi) * sig / (2.0 * dz)
    pref = 1.0 / np.sqrt(2.0 * np.pi * sig * sig)
    scale = f32(cmag * pref / (VEL_UP * IMG_UP * IMG_UP))

    out_half = np.empty((N_GROUPS, 16, 32), dtype=f32)
    for core, r in enumerate(results):
        S = np.asarray(r["out"]).reshape(N_GROUPS, 2, 32, 8)
        pooled = S.sum(axis=3, dtype=np.float64) * scale
        out_half[:, 2 * core:2 * core + 2, :] = pooled.astype(f32)
    full = np.empty((N_GROUPS, 32, 32), dtype=f32)
    full[:, :16, :] = out_half
    full[:, 16:, :] = out_half[::-1, ::-1, ::-1]
    return full


def kernel(inclination, sky_rot, line_broadening):
    in_maps = _host_inputs(inclination, sky_rot, line_broadening)
    res = _run(in_maps)
    return _assemble(res.results, line_broadening)
